# revision 16
# baseline (speedup 1.0000x reference)
"""MoE expert-MLP (SwiGLU) kernel for 8 Trainium2 NeuronCores.

Strategy: expert-parallel. Each of the 8 cores owns one expert's weights.
Routing slots are deduplicated on the host (a token whose K=2 picks hit the
same expert becomes ONE slot with summed weight — exactly matching the
reference's scatter-add), then dispatched per expert and padded to a fixed
capacity. Each core runs a dense [cap, D] SwiGLU MLP in bf16 (1 cycle/row
on the PE array, same rate as fp32r, half the DMA/SBUF) and scales rows by
the routing weight. The host scatter-combines the per-token contributions.

Per-core kernel: ALL weights (Wg, Wu, Wd — 17.3 MB bf16) are SBUF-resident,
DMA'd once at kernel start, so passes only stream x tiles (double-buffered)
and write output. Token passes of 512 keep every matmul's moving dim at 512
(PSUM-bank maximum), amortizing the ~13 ns per-matmul issue overhead.
  stage A: h^T[h, t] = silu(Wg @ x^T) * (Wu @ x^T)   (PSUM accum over D)
  stage B: y[t, d]  = (h^T)^T @ Wd^T, row-scaled by routing weight
"""

import sys
import os

sys.path.insert(0, "/opt/trn_rl_repo")

import numpy as np

T, D, H, E, K = 8192, 2048, 1408, 8, 2
P = 128
HT = H // P        # 11 h-tiles
KT = D // P        # 16 d-tiles
TC = 512           # tokens per pass (= PSUM bank moving-dim max)

_built = {}


def _pass_sizes(cap):
    """Token-pass sizes. The first passes are small so the PE's weight
    consumption in pass 0 doesn't outrun the Wg/Wu DMA streams."""
    lead = []
    for cand in (256, 384):
        if cap - sum(lead) - cand >= 256:
            lead.append(cand)
    rem = cap - sum(lead)
    sizes = lead + [TC] * (rem // TC)
    r = rem % TC
    if r:
        sizes.append(r)
    return sizes


def _cdiv(a, b):
    return -(-a // b)


def _build_nc(cap):
    import concourse.bass as bass  # noqa: F401
    from concourse import bacc
    import concourse.mybir as mybir
    import concourse.tile as tile

    F32 = mybir.dt.float32
    BF16 = mybir.dt.bfloat16
    Silu = mybir.ActivationFunctionType.Silu
    Copy = mybir.ActivationFunctionType.Copy
    Mult = mybir.AluOpType.mult

    sizes = _pass_sizes(cap)
    NP = len(sizes)
    capP = _cdiv(cap, P) * P

    nc = bacc.Bacc("TRN2", target_bir_lowering=False, debug=False)
    xT = nc.declare_dram_parameter("xT", [NP, KT, P, TC], BF16, isOutput=False)
    wg = nc.declare_dram_parameter("wg", [HT, P, KT * P], BF16, isOutput=False)
    wu = nc.declare_dram_parameter("wu", [HT, P, KT * P], BF16, isOutput=False)
    wd = nc.declare_dram_parameter("wd", [HT, P, D], BF16, isOutput=False)
    wt = nc.declare_dram_parameter("wt", [capP], F32, isOutput=False)
    out = nc.declare_dram_parameter("out", [capP, D], BF16, isOutput=True)

    with tile.TileContext(nc) as tc:
        with (
            tc.tile_pool(name="sbuf", bufs=1) as pool,
            tc.tile_pool(name="psum", bufs=1, space="PSUM") as pp,
        ):
            # ---- resident weights, loaded once ----
            # Wg streams on the sync queue, Wu on the scalar queue, x/wt on
            # the gpsimd queue. First h-tiles split into quarters/halves so
            # the first matmuls start as soon as possible. Wd issues are
            # interleaved into pass-0 stage A on the scalar queue (needed
            # only by stage B, keeps them off the critical path).
            wg_ts, wu_ts, wd_ts = [], [], []
            for ht in range(HT):
                wg_1 = pool.tile([P, KT * P], BF16, tag=f"wg{ht}", bufs=1,
                                 name=f"wg{ht}")
                wu_1 = pool.tile([P, KT * P], BF16, tag=f"wu{ht}", bufs=1,
                                 name=f"wu{ht}")
                nsplit = 4 if ht == 0 else (2 if ht == 1 else 1)
                step = KT * P // nsplit
                for q in range(nsplit):
                    sl = slice(q * step, (q + 1) * step)
                    nc.sync.dma_start(wg_1[:, sl], wg[ht, :, sl])
                    nc.scalar.dma_start(wu_1[:, sl], wu[ht, :, sl])
                wg_ts.append(wg_1)
                wu_ts.append(wu_1)
            for ht in range(HT):
                wd_1 = pool.tile([P, D], BF16, tag=f"wd{ht}", bufs=1,
                                 name=f"wd{ht}")
                wd_ts.append(wd_1)

            wt_t = pool.tile([P, capP // P], F32, tag="wt", bufs=1)

            # ---- stage A for ALL passes: h^T = silu(g^T) * u^T ----
            # h for every token stays SBUF-resident, so stage B afterwards
            # is a pure matmul stream with no DMA dependencies at all.
            h_t = pool.tile([P, HT, cap], BF16, tag="ht", bufs=1)
            t0 = 0
            for pi, sz in enumerate(sizes):
                # x^T for this pass, one tile per d-chunk. bufs=1: the WAR
                # dependency on pass p-1's last reader self-throttles the
                # prefetch so it never steals DMA bandwidth from the
                # startup weight streams.
                xt_ts = []
                for dti in range(KT):
                    xt_1 = pool.tile([P, TC], BF16, tag=f"xt{dti}", bufs=1,
                                     name=f"xt{dti}")
                    nc.gpsimd.dma_start(xt_1[:], xT[pi, dti, :, :])
                    xt_ts.append(xt_1)
                if pi == 0:
                    # wt is first needed by stage B
                    nc.gpsimd.dma_start(
                        wt_t[:], wt.rearrange("(n p) -> p n", p=P)
                    )
                wd_pass = min(1, NP - 1)
                for ht in range(HT):
                    psg = pp.tile([P, TC], F32, tag="g", bufs=2, name="psg")
                    for dd in range(KT):
                        nc.tensor.matmul(
                            psg[:, :sz],
                            wg_ts[ht][:, dd * P : (dd + 1) * P],
                            xt_ts[dd][:, :sz],
                            start=(dd == 0),
                            stop=(dd == KT - 1),
                        )
                    st = pool.tile([P, TC], F32, tag="silu", bufs=2, name="st")
                    nc.scalar.activation(st[:, :sz], psg[:, :sz], Silu)
                    psu = pp.tile([P, TC], F32, tag="u", bufs=2, name="psu")
                    for dd in range(KT):
                        nc.tensor.matmul(
                            psu[:, :sz],
                            wu_ts[ht][:, dd * P : (dd + 1) * P],
                            xt_ts[dd][:, :sz],
                            start=(dd == 0),
                            stop=(dd == KT - 1),
                        )
                    nc.vector.tensor_tensor(
                        h_t[:, ht, t0 : t0 + sz], st[:, :sz], psu[:, :sz],
                        op=Mult,
                    )
                    if pi == wd_pass:
                        # slot one Wd stream issue into each h-tile's scalar
                        # idle gap; the startup weight streams are done by
                        # then, and stage B is far away
                        nc.scalar.dma_start(wd_ts[ht][:], wd[ht, :, :])
                t0 += sz

            # ---- stage B: y = h @ Wd^T, scaled by routing weight ----
            # dc-major per token tile: each PSUM bank finishes 11 matmuls
            # before the next starts, so its scale+store overlap the stream
            # and the kernel tail after the very last matmul is one bank.
            NT = _cdiv(cap, P)
            for ts_ in range(NT):
                tw = min(P, cap - ts_ * P)
                y_t = pool.tile([P, D], BF16, tag="yout", bufs=2)
                for dc in range(4):
                    psy = pp.tile([P, TC], F32, tag=f"y{dc}", bufs=1,
                                  name=f"psy{dc}")
                    for ht in range(HT):
                        nc.tensor.matmul(
                            psy[:tw, :],
                            h_t[:, ht, ts_ * P : ts_ * P + tw],
                            wd_ts[ht][:, dc * TC : (dc + 1) * TC],
                            start=(ht == 0),
                            stop=(ht == HT - 1),
                        )
                    dsl = slice(dc * TC, (dc + 1) * TC)
                    if dc % 2 == 0:
                        nc.vector.tensor_scalar_mul(
                            y_t[:tw, dsl],
                            psy[:tw, :],
                            wt_t[:tw, ts_ : ts_ + 1],
                        )
                    else:
                        nc.scalar.activation(
                            y_t[:tw, dsl],
                            psy[:tw, :],
                            Copy,
                            scale=wt_t[:tw, ts_ : ts_ + 1],
                        )
                    # spread the final tile's stores across queues so the
                    # last one isn't stuck behind serialized sync issues
                    if ts_ == NT - 1:
                        eng = (nc.sync, nc.scalar, nc.gpsimd, nc.sync)[dc]
                    else:
                        eng = nc.sync
                    eng.dma_start(
                        out[ts_ * P : ts_ * P + tw, dsl], y_t[:tw, dsl]
                    )

    nc.finalize()
    return nc


def _get_nc(cap):
    if cap not in _built:
        _built[cap] = _build_nc(cap)
    return _built[cap]


def kernel(x, weights, Wg, Wu, Wd, indices, seq_len=None, **_unused):
    from concourse.bass_utils import run_bass_kernel_spmd
    import ml_dtypes

    BF = ml_dtypes.bfloat16

    x = np.asarray(x, dtype=np.float32)
    weights = np.asarray(weights, dtype=np.float32)
    Wg = np.asarray(Wg, dtype=np.float32)
    Wu = np.asarray(Wu, dtype=np.float32)
    Wd = np.asarray(Wd, dtype=np.float32)
    idx = np.asarray(indices).astype(np.int64)

    t, d = x.shape
    e, h, _ = Wg.shape

    # ---- host-side routing (dispatch), with same-expert slot dedup ----
    # A token routed twice to the same expert contributes once with summed
    # weight (matches the reference's scatter-add coefficient).
    e0, e1 = idx[:, 0], idx[:, 1]
    dup = e0 == e1
    tok_all = np.concatenate([np.arange(t, dtype=np.int64),
                              np.arange(t, dtype=np.int64)[~dup]])
    exp_all = np.concatenate([e0, e1[~dup]])
    w_all = np.concatenate([
        np.where(dup, weights[:, 0] + weights[:, 1], weights[:, 0]),
        weights[~dup, 1],
    ]).astype(np.float32)
    nslots = tok_all.shape[0]

    order = np.argsort(exp_all, kind="stable")
    counts = np.bincount(exp_all, minlength=e)
    starts = np.zeros(e + 1, dtype=np.int64)
    starts[1:] = np.cumsum(counts)
    cap = max(int(counts.max()), 512)
    sizes = _pass_sizes(cap)
    np_ = len(sizes)
    capP = _cdiv(cap, P) * P

    tok_sorted = tok_all[order]
    w_sorted = w_all[order]

    xb = x.astype(BF)
    in_maps = []
    for ei in range(e):
        n = int(counts[ei])
        toks = tok_sorted[starts[ei] : starts[ei] + n]
        # pass-major x^T: xTp[pi][dti][p][tc] = x[tok(pass_off[pi]+tc), dti*P+p]
        xe = np.zeros((cap, d), dtype=BF)
        xe[:n] = xb[toks]
        xTp = np.zeros((np_, KT, P, TC), dtype=BF)
        off = 0
        for pi, sz in enumerate(sizes):
            blk = xe[off : off + sz]  # [sz, d]
            xTp[pi, :, :, :sz] = blk.reshape(sz, KT, P).transpose(1, 2, 0)
            off += sz
        wvec = np.zeros(capP, dtype=np.float32)
        wvec[:n] = w_sorted[starts[ei] : starts[ei] + n]
        # pack Wg/Wu so each h-tile block is one contiguous [128, 2048] DMA:
        # block[ht][p][k*128+hh] = Wg[e].T[k*128+p, ht*128+hh]
        WgT = Wg[ei].T  # [D, H]
        WuT = Wu[ei].T
        wg_lin = np.ascontiguousarray(
            WgT.reshape(KT, P, HT, P).transpose(2, 1, 0, 3).reshape(HT, P, KT * P),
            dtype=BF,
        )
        wu_lin = np.ascontiguousarray(
            WuT.reshape(KT, P, HT, P).transpose(2, 1, 0, 3).reshape(HT, P, KT * P),
            dtype=BF,
        )
        wd_lin = np.ascontiguousarray(
            Wd[ei].T.reshape(HT, P, D), dtype=BF
        )
        in_maps.append(
            {
                "xT": xTp,
                "wg": wg_lin,
                "wu": wu_lin,
                "wd": wd_lin,
                "wt": wvec,
            }
        )

    nc = _get_nc(cap)
    trace = bool(int(os.environ.get("KERNEL_TRACE", "0")))
    res = run_bass_kernel_spmd(
        nc, in_maps, core_ids=list(range(e)), trace=trace
    )
    if trace:
        kernel.last_exec_time_ns = res.exec_time_ns
        kernel.last_results = res

    # ---- host-side combine ----
    allres = np.concatenate(
        [res.results[ei]["out"][: counts[ei]] for ei in range(e)], axis=0
    ).astype(np.float32)
    inv = np.empty(nslots, dtype=np.int64)
    inv[order] = np.arange(nslots, dtype=np.int64)
    padded = np.concatenate([allres, np.zeros((1, d), np.float32)], axis=0)
    pos0 = inv[:t]
    pos1 = np.full(t, nslots, dtype=np.int64)
    pos1[np.arange(t)[~dup]] = inv[t:]
    y = padded[pos0] + padded[pos1]
    return y


# revision 18
# speedup vs baseline: 1.0460x; 1.0460x over previous
"""MoE expert-MLP (SwiGLU) kernel for 8 Trainium2 NeuronCores.

Strategy: expert-parallel. Each of the 8 cores owns one expert's weights.
Routing slots are deduplicated on the host (a token whose K=2 picks hit the
same expert becomes ONE slot with summed weight — exactly matching the
reference's scatter-add), then dispatched per expert and padded to a fixed
capacity. Each core runs a dense [cap, D] SwiGLU MLP in bf16 (1 cycle/row
on the PE array, same rate as fp32r, half the DMA/SBUF) and scales rows by
the routing weight. The host scatter-combines the per-token contributions.

Per-core kernel: ALL weights (Wg, Wu, Wd — 17.3 MB bf16) are SBUF-resident,
DMA'd once at kernel start, so passes only stream x tiles (double-buffered)
and write output. Token passes of 512 keep every matmul's moving dim at 512
(PSUM-bank maximum), amortizing the ~13 ns per-matmul issue overhead.
  stage A: h^T[h, t] = silu(Wg @ x^T) * (Wu @ x^T)   (PSUM accum over D)
  stage B: y[t, d]  = (h^T)^T @ Wd^T, row-scaled by routing weight
"""

import sys
import os

sys.path.insert(0, "/opt/trn_rl_repo")

import numpy as np

T, D, H, E, K = 8192, 2048, 1408, 8, 2
P = 128
HT = H // P        # 11 h-tiles
KT = D // P        # 16 d-tiles
TC = 512           # tokens per pass (= PSUM bank moving-dim max)

_built = {}


def _pass_sizes(cap):
    """Near-equal token-pass sizes (each <= TC, no tiny trailing pass)."""
    np_ = max(2, _cdiv(cap, TC))
    base = cap // np_
    rem = cap - base * np_
    return [base + (1 if i < rem else 0) for i in range(np_)]


def _cdiv(a, b):
    return -(-a // b)


def _build_nc(cap):
    import concourse.bass as bass  # noqa: F401
    from concourse import bacc
    import concourse.mybir as mybir
    import concourse.tile as tile

    F32 = mybir.dt.float32
    BF16 = mybir.dt.bfloat16
    Silu = mybir.ActivationFunctionType.Silu
    Copy = mybir.ActivationFunctionType.Copy
    Mult = mybir.AluOpType.mult

    sizes = _pass_sizes(cap)
    NP = len(sizes)
    capP = _cdiv(cap, P) * P

    nc = bacc.Bacc("TRN2", target_bir_lowering=False, debug=False)
    xT = nc.declare_dram_parameter("xT", [NP, KT, P, TC], BF16, isOutput=False)
    wg = nc.declare_dram_parameter("wg", [HT, P, KT * P], BF16, isOutput=False)
    wu = nc.declare_dram_parameter("wu", [HT, P, KT * P], BF16, isOutput=False)
    wd = nc.declare_dram_parameter("wd", [HT, P, D], BF16, isOutput=False)
    wt = nc.declare_dram_parameter("wt", [capP], F32, isOutput=False)
    out = nc.declare_dram_parameter("out", [capP, D], BF16, isOutput=True)

    with tile.TileContext(nc) as tc:
        with (
            tc.tile_pool(name="sbuf", bufs=1) as pool,
            tc.tile_pool(name="psum", bufs=1, space="PSUM") as pp,
        ):
            # ---- resident weights, loaded once ----
            # Wg streams on the sync queue, Wu on the scalar queue, x/wt on
            # the gpsimd queue. First h-tiles split into quarters/halves so
            # the first matmuls start as soon as possible. Wd issues are
            # interleaved into pass-0 stage A on the scalar queue (needed
            # only by stage B, keeps them off the critical path).
            wg_ts, wu_ts, wd_ts = [], [], []
            for ht in range(HT):
                wg_1 = pool.tile([P, KT * P], BF16, tag=f"wg{ht}", bufs=1,
                                 name=f"wg{ht}")
                wu_1 = pool.tile([P, KT * P], BF16, tag=f"wu{ht}", bufs=1,
                                 name=f"wu{ht}")
                nsplit = 4 if ht == 0 else (2 if ht == 1 else 1)
                step = KT * P // nsplit
                for q in range(nsplit):
                    sl = slice(q * step, (q + 1) * step)
                    nc.sync.dma_start(wg_1[:, sl], wg[ht, :, sl])
                    nc.scalar.dma_start(wu_1[:, sl], wu[ht, :, sl])
                wg_ts.append(wg_1)
                wu_ts.append(wu_1)
            for ht in range(HT):
                wd_1 = pool.tile([P, D], BF16, tag=f"wd{ht}", bufs=1,
                                 name=f"wd{ht}")
                wd_ts.append(wd_1)

            wt_t = pool.tile([P, capP // P], F32, tag="wt", bufs=1)

            # ---- software-pipelined schedule ----
            # h for every token stays SBUF-resident. Stage-B token tiles
            # are interposed between consecutive stage-A passes: they give
            # the next pass's x-prefetch (bufs=1, WAR-throttled) a DMA
            # window while keeping the PE streaming. The remaining stage-B
            # tiles form a pure-stream tail with no DMA dependencies.
            h_t = pool.tile([P, HT, cap], BF16, tag="ht", bufs=1)
            NT = _cdiv(cap, P)
            csum = [0]
            for sz in sizes:
                csum.append(csum[-1] + sz)

            def stage_a(pi, sz, t0):
                xt_ts = []
                for dti in range(KT):
                    xt_1 = pool.tile([P, TC], BF16, tag=f"xt{dti}", bufs=1,
                                     name=f"xt{dti}")
                    nc.gpsimd.dma_start(xt_1[:], xT[pi, dti, :, :])
                    xt_ts.append(xt_1)
                if pi == 0:
                    # wt is first needed by stage B
                    nc.gpsimd.dma_start(
                        wt_t[:], wt.rearrange("(n p) -> p n", p=P)
                    )
                # Wd must be resident before the first interposed B tile
                # (right after pass 0); the 495-token passes leave enough
                # DMA margin to stream it alongside Wg/Wu.
                wd_pass = 0
                for ht in range(HT):
                    psg = pp.tile([P, TC], F32, tag="g", bufs=2, name="psg")
                    for dd in range(KT):
                        nc.tensor.matmul(
                            psg[:, :sz],
                            wg_ts[ht][:, dd * P : (dd + 1) * P],
                            xt_ts[dd][:, :sz],
                            start=(dd == 0),
                            stop=(dd == KT - 1),
                        )
                    st = pool.tile([P, TC], F32, tag="silu", bufs=2, name="st")
                    nc.scalar.activation(st[:, :sz], psg[:, :sz], Silu)
                    psu = pp.tile([P, TC], F32, tag="u", bufs=2, name="psu")
                    for dd in range(KT):
                        nc.tensor.matmul(
                            psu[:, :sz],
                            wu_ts[ht][:, dd * P : (dd + 1) * P],
                            xt_ts[dd][:, :sz],
                            start=(dd == 0),
                            stop=(dd == KT - 1),
                        )
                    nc.vector.tensor_tensor(
                        h_t[:, ht, t0 : t0 + sz], st[:, :sz], psu[:, :sz],
                        op=Mult,
                    )
                    if pi == wd_pass:
                        # one Wd stream issue per h-tile on the scalar
                        # queue: after the startup streams, before stage B
                        nc.scalar.dma_start(wd_ts[ht][:], wd[ht, :, :])

            def stage_b_tile(ts_):
                # dc-major: each PSUM bank finishes its 11-matmul chain
                # before the next starts, so scale+store overlap the stream
                # and the tail after the very last matmul is one bank.
                tw = min(P, cap - ts_ * P)
                y_t = pool.tile([P, D], BF16, tag="yout", bufs=2)
                for dc in range(4):
                    psy = pp.tile([P, TC], F32, tag=f"y{dc}", bufs=1,
                                  name=f"psy{dc}")
                    for ht in range(HT):
                        nc.tensor.matmul(
                            psy[:tw, :],
                            h_t[:, ht, ts_ * P : ts_ * P + tw],
                            wd_ts[ht][:, dc * TC : (dc + 1) * TC],
                            start=(ht == 0),
                            stop=(ht == HT - 1),
                        )
                    dsl = slice(dc * TC, (dc + 1) * TC)
                    if dc % 2 == 0:
                        nc.vector.tensor_scalar_mul(
                            y_t[:tw, dsl],
                            psy[:tw, :],
                            wt_t[:tw, ts_ : ts_ + 1],
                        )
                    else:
                        nc.scalar.activation(
                            y_t[:tw, dsl],
                            psy[:tw, :],
                            Copy,
                            scale=wt_t[:tw, ts_ : ts_ + 1],
                        )
                    # spread the final tile's stores across queues so the
                    # last one isn't stuck behind serialized sync issues
                    if ts_ == NT - 1:
                        eng = (nc.sync, nc.scalar, nc.gpsimd, nc.sync)[dc]
                    else:
                        eng = nc.sync
                    eng.dma_start(
                        out[ts_ * P : ts_ * P + tw, dsl], y_t[:tw, dsl]
                    )

            b_next = 0
            for pi, sz in enumerate(sizes):
                stage_a(pi, sz, csum[pi])
                if pi < NP - 1:
                    # interpose B tiles (only ones whose h is complete)
                    want = 3 if pi == 0 else 4
                    avail = csum[pi + 1] // P
                    while want > 0 and b_next < avail:
                        stage_b_tile(b_next)
                        b_next += 1
                        want -= 1
            while b_next < NT:
                stage_b_tile(b_next)
                b_next += 1

    nc.finalize()
    return nc


def _get_nc(cap):
    if cap not in _built:
        _built[cap] = _build_nc(cap)
    return _built[cap]


def kernel(x, weights, Wg, Wu, Wd, indices, seq_len=None, **_unused):
    from concourse.bass_utils import run_bass_kernel_spmd
    import ml_dtypes

    BF = ml_dtypes.bfloat16

    x = np.asarray(x, dtype=np.float32)
    weights = np.asarray(weights, dtype=np.float32)
    Wg = np.asarray(Wg, dtype=np.float32)
    Wu = np.asarray(Wu, dtype=np.float32)
    Wd = np.asarray(Wd, dtype=np.float32)
    idx = np.asarray(indices).astype(np.int64)

    t, d = x.shape
    e, h, _ = Wg.shape

    # ---- host-side routing (dispatch), with same-expert slot dedup ----
    # A token routed twice to the same expert contributes once with summed
    # weight (matches the reference's scatter-add coefficient).
    e0, e1 = idx[:, 0], idx[:, 1]
    dup = e0 == e1
    tok_all = np.concatenate([np.arange(t, dtype=np.int64),
                              np.arange(t, dtype=np.int64)[~dup]])
    exp_all = np.concatenate([e0, e1[~dup]])
    w_all = np.concatenate([
        np.where(dup, weights[:, 0] + weights[:, 1], weights[:, 0]),
        weights[~dup, 1],
    ]).astype(np.float32)
    nslots = tok_all.shape[0]

    order = np.argsort(exp_all, kind="stable")
    counts = np.bincount(exp_all, minlength=e)
    starts = np.zeros(e + 1, dtype=np.int64)
    starts[1:] = np.cumsum(counts)
    cap = max(int(counts.max()), 512)
    sizes = _pass_sizes(cap)
    np_ = len(sizes)
    capP = _cdiv(cap, P) * P

    tok_sorted = tok_all[order]
    w_sorted = w_all[order]

    xb = x.astype(BF)
    in_maps = []
    for ei in range(e):
        n = int(counts[ei])
        toks = tok_sorted[starts[ei] : starts[ei] + n]
        # pass-major x^T: xTp[pi][dti][p][tc] = x[tok(pass_off[pi]+tc), dti*P+p]
        xe = np.zeros((cap, d), dtype=BF)
        xe[:n] = xb[toks]
        xTp = np.zeros((np_, KT, P, TC), dtype=BF)
        off = 0
        for pi, sz in enumerate(sizes):
            blk = xe[off : off + sz]  # [sz, d]
            xTp[pi, :, :, :sz] = blk.reshape(sz, KT, P).transpose(1, 2, 0)
            off += sz
        wvec = np.zeros(capP, dtype=np.float32)
        wvec[:n] = w_sorted[starts[ei] : starts[ei] + n]
        # pack Wg/Wu so each h-tile block is one contiguous [128, 2048] DMA:
        # block[ht][p][k*128+hh] = Wg[e].T[k*128+p, ht*128+hh]
        WgT = Wg[ei].T  # [D, H]
        WuT = Wu[ei].T
        wg_lin = np.ascontiguousarray(
            WgT.reshape(KT, P, HT, P).transpose(2, 1, 0, 3).reshape(HT, P, KT * P),
            dtype=BF,
        )
        wu_lin = np.ascontiguousarray(
            WuT.reshape(KT, P, HT, P).transpose(2, 1, 0, 3).reshape(HT, P, KT * P),
            dtype=BF,
        )
        wd_lin = np.ascontiguousarray(
            Wd[ei].T.reshape(HT, P, D), dtype=BF
        )
        in_maps.append(
            {
                "xT": xTp,
                "wg": wg_lin,
                "wu": wu_lin,
                "wd": wd_lin,
                "wt": wvec,
            }
        )

    nc = _get_nc(cap)
    trace = bool(int(os.environ.get("KERNEL_TRACE", "0")))
    res = run_bass_kernel_spmd(
        nc, in_maps, core_ids=list(range(e)), trace=trace
    )
    if trace:
        kernel.last_exec_time_ns = res.exec_time_ns
        kernel.last_results = res

    # ---- host-side combine ----
    allres = np.concatenate(
        [res.results[ei]["out"][: counts[ei]] for ei in range(e)], axis=0
    ).astype(np.float32)
    inv = np.empty(nslots, dtype=np.int64)
    inv[order] = np.arange(nslots, dtype=np.int64)
    padded = np.concatenate([allres, np.zeros((1, d), np.float32)], axis=0)
    pos0 = inv[:t]
    pos1 = np.full(t, nslots, dtype=np.int64)
    pos1[np.arange(t)[~dup]] = inv[t:]
    y = padded[pos0] + padded[pos1]
    return y
